# revision 1
# baseline (speedup 1.0000x reference)
"""AttentionBlock kernel for 8 Trainium2 NeuronCores.

Sharding: core c -> batch b = c//2, parity p = c%2. Each core computes the
transformer block for query tiles {i : i%2 == p} (8 tiles of 128 rows) of
batch b. Causal attention work is balanced: slot j (global tile 2j+p) uses
context (j+1)*256, identical across parities, so one SPMD program serves all
8 cores. K/V are computed on-core from the full batch context (no
cross-core communication).

Dtypes: LN/residual/FFN matmuls in float32r (tf32-like, full-rate); h/Q/K/V
and attention probabilities in bf16. PSUM accumulation is fp32 throughout.
"""
import sys
sys.path.insert(0, "/opt/trn_rl_repo")

import numpy as np
import ml_dtypes

import concourse.bacc as bacc
import concourse.bass as bass
import concourse.mybir as mybir
import concourse.tile as tile
from concourse import bass_utils
from concourse.masks import make_identity

P = 128
F32 = mybir.dt.float32
F32R = mybir.dt.float32r
BF16 = mybir.dt.bfloat16

FULL = dict(T=2048, D=2048, H=16, FF=8192)
SMALL = dict(T=1536, D=512, H=4, FF=2048)


def build_nc(cfg):
    T, D, H, FF = cfg["T"], cfg["D"], cfg["H"], cfg["FF"]
    HD = 128
    NT = T // P          # context tiles
    NQ = NT // 2         # query slots per core
    TQ = NQ * P          # query rows per core
    ND = D // P          # d tiles
    NF = FF // P         # ffn hidden tiles
    FGN = 16 if NF % 16 == 0 else 4   # f tiles per group
    NG = NF // FGN
    HG = 4               # heads per V-production group
    SCALE = 1.0 / np.sqrt(HD)
    EPS = 1e-5

    nc = bacc.Bacc("TRN2", target_bir_lowering=False)

    x_ctx = nc.dram_tensor("x_ctx", [T, D], F32, kind="ExternalInput")
    xq = nc.dram_tensor("xq", [TQ, D], F32, kind="ExternalInput")
    Wq = nc.dram_tensor("Wq", [H, D, HD], BF16, kind="ExternalInput")
    Wk = nc.dram_tensor("Wk", [H, D, HD], BF16, kind="ExternalInput")
    Wv = nc.dram_tensor("Wv", [H, D, HD], BF16, kind="ExternalInput")
    bq = nc.dram_tensor("bq", [H, HD], F32, kind="ExternalInput")
    bk = nc.dram_tensor("bk", [H, HD], F32, kind="ExternalInput")
    bv = nc.dram_tensor("bv", [H, HD], F32, kind="ExternalInput")
    W1 = nc.dram_tensor("W1", [D, FF], F32R, kind="ExternalInput")
    b1 = nc.dram_tensor("b1", [FF], F32, kind="ExternalInput")
    W2 = nc.dram_tensor("W2", [FF, D], BF16, kind="ExternalInput")
    b2 = nc.dram_tensor("b2", [D], F32, kind="ExternalInput")
    g1 = nc.dram_tensor("g1", [D], F32, kind="ExternalInput")
    be1 = nc.dram_tensor("be1", [D], F32, kind="ExternalInput")
    g2 = nc.dram_tensor("g2", [D], F32, kind="ExternalInput")
    be2 = nc.dram_tensor("be2", [D], F32, kind="ExternalInput")
    mask = nc.dram_tensor("mask", [P, 256], F32, kind="ExternalInput")
    out = nc.dram_tensor("out", [TQ, D], F32, kind="ExternalOutput")

    def colsplit(v):
        # [D] dram vector -> sbuf [128, ND] (col d = slice of tile d)
        return bass.AP(tensor=v.ap().tensor, offset=0, ap=[[1, P], [P, ND]])

    with tile.TileContext(nc) as tc:
        ID = mybir.ActivationFunctionType.Identity
        EXP = mybir.ActivationFunctionType.Exp
        RELU = mybir.ActivationFunctionType.Relu
        SQRT = mybir.ActivationFunctionType.Sqrt

        with tc.tile_pool(name="consts", bufs=1) as consts, \
             tc.tile_pool(name="dram", bufs=1, space="DRAM") as dpool:
            ident_bf = consts.tile([P, P], BF16)
            make_identity(nc, ident_bf)
            ident_fr = consts.tile([P, P], F32, tag="identfr")
            make_identity(nc, ident_fr)
            eps_t = consts.tile([P, 1], F32)
            nc.gpsimd.memset(eps_t, EPS)
            g1c = consts.tile([P, ND], F32, tag="g1c")
            nc.sync.dma_start(out=g1c, in_=colsplit(g1))
            be1c = consts.tile([P, ND], F32, tag="be1c")
            nc.sync.dma_start(out=be1c, in_=colsplit(be1))
            g2c = consts.tile([P, ND], F32, tag="g2c")
            nc.sync.dma_start(out=g2c, in_=colsplit(g2))
            be2c = consts.tile([P, ND], F32, tag="be2c")
            nc.sync.dma_start(out=be2c, in_=colsplit(be2))
            # bq/bk: [H, HD] -> [128(e), H]
            bqc = consts.tile([P, H], F32, tag="bqc")
            nc.sync.dma_start(out=bqc, in_=bass.AP(
                tensor=bq.ap().tensor, offset=0, ap=[[1, P], [HD, H]]))
            bkc = consts.tile([P, H], F32, tag="bkc")
            nc.sync.dma_start(out=bkc, in_=bass.AP(
                tensor=bk.ap().tensor, offset=0, ap=[[1, P], [HD, H]]))
            b1c = consts.tile([P, NF], F32, tag="b1c")
            nc.sync.dma_start(out=b1c, in_=bass.AP(
                tensor=b1.ap().tensor, offset=0, ap=[[1, P], [P, NF]]))
            mask_t = consts.tile([P, 256], F32, tag="maskt")
            nc.sync.dma_start(out=mask_t, in_=mask.ap())
            b2b = consts.tile([P, D], F32, tag="b2b")
            nc.sync.dma_start(out=b2b, in_=bass.AP(
                tensor=b2.ap().tensor, offset=0, ap=[[0, P], [1, D]]))

            attn_dram = dpool.tile([TQ, D], F32, tag="attn_dram")
            x2_dram = dpool.tile([TQ, D], F32, tag="x2_dram")
            ff_dram = [dpool.tile([TQ, D], F32, tag=f"ff{g}", name=f"ff{g}") for g in range(NG)]

            # ============ Phase A: LN1 + transpose -> hT (bf16) ============
            with tc.tile_pool(name="hT", bufs=1) as hTp:
                hTs = [hTp.tile([P, T + TQ], BF16, tag=f"hT{d}", name=f"hT{d}") for d in range(ND)]

                with tc.tile_pool(name="phA", bufs=3) as pa, \
                     tc.tile_pool(name="phA2", bufs=2) as pa2, \
                     tc.tile_pool(name="psA", bufs=4, space="PSUM") as psA:
                    def ln1_tile(src_ap, dst_col):
                        xin = pa.tile([P, D], F32, tag="xin")
                        nc.sync.dma_start(out=xin, in_=src_ap)
                        nsub = max(1, D // 512)
                        st = pa.tile([P, nsub, 6], F32, tag="st")
                        xr = xin.rearrange("p (n f) -> p n f", n=nsub)
                        for s in range(nsub):
                            nc.vector.bn_stats(out=st[:, s, :], in_=xr[:, s, :])
                        mv = pa.tile([P, 2], F32, tag="mv")
                        nc.vector.bn_aggr(out=mv, in_=st)
                        rstd = pa.tile([P, 1], F32, tag="rstd")
                        nc.scalar.activation(out=rstd, in_=mv[:, 1:2], func=SQRT,
                                             bias=eps_t, scale=1.0)
                        nc.vector.reciprocal(out=rstd, in_=rstd)
                        hb = pa2.tile([P, D], BF16, tag="hb")
                        nc.vector.tensor_scalar(
                            out=hb, in0=xin, scalar1=mv[:, 0:1], scalar2=rstd,
                            op0=mybir.AluOpType.subtract, op1=mybir.AluOpType.mult)
                        for d in range(ND):
                            tp = psA.tile([P, P], BF16, tag="tpA")
                            nc.tensor.transpose(tp, hb[:, d * P:(d + 1) * P], ident_bf)
                            nc.scalar.activation(
                                out=hTs[d][:, dst_col:dst_col + P], in_=tp, func=ID,
                                bias=be1c[:, d:d + 1], scale=g1c[:, d:d + 1])

                    xr_ctx = x_ctx.ap().rearrange("(n p) d -> n p d", p=P)
                    for g in range(NT):
                        ln1_tile(xr_ctx[g], g * P)
                    xr_q = xq.ap().rearrange("(n p) d -> n p d", p=P)
                    for j in range(NQ):
                        ln1_tile(xr_q[j], T + j * P)

                # ============ Phase B: QKV + attention per head ============
                with tc.tile_pool(name="phB", bufs=2) as pb, \
                     tc.tile_pool(name="phBv", bufs=1) as pbv, \
                     tc.tile_pool(name="phBs", bufs=4) as pbs, \
                     tc.tile_pool(name="psL", bufs=2, space="PSUM") as psL, \
                     tc.tile_pool(name="ps512", bufs=2, space="PSUM") as ps512, \
                     tc.tile_pool(name="psT", bufs=2, space="PSUM") as psT, \
                     tc.tile_pool(name="psAV", bufs=2, space="PSUM") as psAV:
                    for h in range(H):
                        hl = h % HG
                        if hl == 0:
                            # --- V for 4 heads: V4buf[s, (hl e)] bf16 ---
                            wv4 = pbv.tile([P, ND, HG * HD], BF16, tag="wv4")
                            for q in range(HG):
                                nc.sync.dma_start(
                                    out=wv4[:, :, q * HD:(q + 1) * HD],
                                    in_=bass.AP(tensor=Wv.ap().tensor,
                                                offset=(h + q) * D * HD,
                                                ap=[[HD, P], [P * HD, ND], [1, HD]]))
                            bv4 = pbs.tile([P, HG * HD], F32, tag="bv4")
                            nc.sync.dma_start(out=bv4, in_=bass.AP(
                                tensor=bv.ap().tensor, offset=h * HD,
                                ap=[[0, P], [1, HG * HD]]))
                            V4 = pbv.tile([P, NT, HG * HD], BF16, tag="V4")
                            for s in range(NT):
                                pv = ps512.tile([P, HG * HD], F32, tag="p512")
                                for d in range(ND):
                                    nc.tensor.matmul(
                                        pv, hTs[d][:, s * P:(s + 1) * P], wv4[:, d, :],
                                        start=(d == 0), stop=(d == ND - 1))
                                nc.vector.tensor_add(out=V4[:, s, :], in0=pv, in1=bv4)
                        # --- QT / KT ---
                        wq_t = pb.tile([P, ND, HD], BF16, tag="wq")
                        nc.sync.dma_start(out=wq_t, in_=bass.AP(
                            tensor=Wq.ap().tensor, offset=h * D * HD,
                            ap=[[HD, P], [P * HD, ND], [1, HD]]))
                        wk_t = pb.tile([P, ND, HD], BF16, tag="wk")
                        nc.sync.dma_start(out=wk_t, in_=bass.AP(
                            tensor=Wk.ap().tensor, offset=h * D * HD,
                            ap=[[HD, P], [P * HD, ND], [1, HD]]))
                        QT = pb.tile([P, TQ], BF16, tag="QT")
                        for c0 in range(0, TQ, 512):
                            cl = min(512, TQ - c0)
                            pq = ps512.tile([P, 512], F32, tag="p512")
                            for d in range(ND):
                                nc.tensor.matmul(
                                    pq[:, :cl], wq_t[:, d, :], hTs[d][:, T + c0:T + c0 + cl],
                                    start=(d == 0), stop=(d == ND - 1))
                            nc.scalar.activation(out=QT[:, c0:c0 + cl],
                                                 in_=pq[:, :cl], func=ID, bias=bqc[:, h:h + 1],
                                                 scale=1.0)
                        KT = pb.tile([P, T], BF16, tag="KT")
                        for c in range(T // 512):
                            pk = ps512.tile([P, 512], F32, tag="p512")
                            for d in range(ND):
                                nc.tensor.matmul(
                                    pk, wk_t[:, d, :], hTs[d][:, c * 512:(c + 1) * 512],
                                    start=(d == 0), stop=(d == ND - 1))
                            nc.scalar.activation(out=KT[:, c * 512:(c + 1) * 512],
                                                 in_=pk, func=ID, bias=bkc[:, h:h + 1],
                                                 scale=1.0)
                        # --- attention slots ---
                        for j in range(NQ):
                            ctx = (j + 1) * 256
                            nst = 2 * (j + 1)
                            attn = pb.tile([P, T], BF16, tag="attn")
                            nrounds = (ctx + 511) // 512
                            sig = pbs.tile([P, max(nrounds, 2)], F32, tag="sig")
                            for r in range(nrounds):
                                off = r * 512
                                rlen = min(ctx - off, 512)
                                lp = psL.tile([P, 512], F32, tag="logits")
                                nc.tensor.matmul(
                                    lp[:, :rlen], QT[:, j * P:(j + 1) * P],
                                    KT[:, off:off + rlen],
                                    start=True, stop=True)
                                if off + rlen == ctx:
                                    nc.vector.tensor_add(
                                        out=lp[:, rlen - 256:rlen],
                                        in0=lp[:, rlen - 256:rlen], in1=mask_t)
                                nc.scalar.activation(
                                    out=attn[:, off:off + rlen], in_=lp[:, :rlen],
                                    func=EXP, scale=SCALE,
                                    accum_out=sig[:, r:r + 1])
                            if nrounds > 1:
                                for r in range(1, nrounds):
                                    nc.vector.tensor_add(out=sig[:, 0:1],
                                                         in0=sig[:, 0:1],
                                                         in1=sig[:, r:r + 1])
                            rs = pbs.tile([P, 1], F32, tag="rs")
                            nc.vector.reciprocal(out=rs, in_=sig[:, 0:1])
                            attnT = pb.tile([P, T], BF16, tag="attnT")
                            for s in range(nst):
                                tp = psT.tile([P, P], BF16, tag="tpB")
                                nc.tensor.transpose(
                                    tp, attn[:, s * P:(s + 1) * P], ident_bf)
                                nc.vector.tensor_copy(
                                    attnT[:, s * P:(s + 1) * P], tp)
                            av = psAV.tile([P, HD], F32, tag="av")
                            for s in range(nst):
                                nc.tensor.matmul(
                                    av, attnT[:, s * P:(s + 1) * P],
                                    V4[:, s, hl * HD:(hl + 1) * HD],
                                    start=(s == 0), stop=(s == nst - 1))
                            ob = pbs.tile([P, HD], F32, tag="ob")
                            nc.vector.tensor_scalar_mul(out=ob, in0=av, scalar1=rs)
                            nc.sync.dma_start(
                                out=attn_dram[j * P:(j + 1) * P, h * HD:(h + 1) * HD],
                                in_=ob)

            # ============ Phase C: residual + LN2 + FFN ============
            with tc.tile_pool(name="h2T", bufs=1) as h2p:
                h2Ts = [h2p.tile([P, TQ], F32R, tag=f"h2T{d}", name=f"h2T{d}") for d in range(ND)]
                with tc.tile_pool(name="phC", bufs=3) as pc, \
                     tc.tile_pool(name="phC2", bufs=2) as pc2, \
                     tc.tile_pool(name="psC", bufs=2, space="PSUM") as psC:
                    xr_q = xq.ap().rearrange("(n p) d -> n p d", p=P)
                    for t in range(NQ):
                        xt = pc.tile([P, D], F32, tag="xt")
                        nc.sync.dma_start(out=xt, in_=xr_q[t])
                        at = pc.tile([P, D], F32, tag="at")
                        nc.sync.dma_start(out=at, in_=attn_dram[t * P:(t + 1) * P, :])
                        x2 = pc.tile([P, D], F32, tag="x2t")
                        nc.vector.tensor_add(out=x2, in0=xt, in1=at)
                        nc.sync.dma_start(out=x2_dram[t * P:(t + 1) * P, :], in_=x2)
                        nsub = max(1, D // 512)
                        st = pc.tile([P, nsub, 6], F32, tag="st2")
                        x2r = x2.rearrange("p (n f) -> p n f", n=nsub)
                        for s in range(nsub):
                            nc.vector.bn_stats(out=st[:, s, :], in_=x2r[:, s, :])
                        mv = pc.tile([P, 2], F32, tag="mv2")
                        nc.vector.bn_aggr(out=mv, in_=st)
                        rstd = pc.tile([P, 1], F32, tag="rstd2")
                        nc.scalar.activation(out=rstd, in_=mv[:, 1:2], func=SQRT,
                                             bias=eps_t, scale=1.0)
                        nc.vector.reciprocal(out=rstd, in_=rstd)
                        h2 = pc2.tile([P, D], F32, tag="h2tmp")
                        nc.vector.tensor_scalar(
                            out=h2, in0=x2, scalar1=mv[:, 0:1], scalar2=rstd,
                            op0=mybir.AluOpType.subtract, op1=mybir.AluOpType.mult)
                        for d in range(ND):
                            tp = psC.tile([P, P], F32, tag="tpC")
                            nc.tensor.transpose(tp, h2[:, d * P:(d + 1) * P], ident_fr)
                            nc.scalar.activation(
                                out=h2Ts[d][:, t * P:(t + 1) * P], in_=tp, func=ID,
                                bias=be2c[:, d:d + 1], scale=g2c[:, d:d + 1])

                with tc.tile_pool(name="phU", bufs=1) as pu, \
                     tc.tile_pool(name="phW1", bufs=3) as pw1, \
                     tc.tile_pool(name="phW2", bufs=2) as pw2, \
                     tc.tile_pool(name="phCb", bufs=4) as pcb, \
                     tc.tile_pool(name="psU", bufs=2, space="PSUM") as psU, \
                     tc.tile_pool(name="psO", bufs=2, space="PSUM") as psO:
                    Us = [pu.tile([P, TQ], BF16, tag=f"u{i}", name=f"u{i}") for i in range(FGN)]
                    for g in range(NG):
                        for fi in range(FGN):
                            f = g * FGN + fi
                            w1f = pw1.tile([P, ND, P], F32R, tag="w1f")
                            nc.sync.dma_start(out=w1f, in_=bass.AP(
                                tensor=W1.ap().tensor, offset=f * P,
                                ap=[[FF, P], [P * FF, ND], [1, P]]))
                            for c0 in range(0, TQ, 512):
                                cl = min(512, TQ - c0)
                                up = psU.tile([P, 512], F32, tag="up")
                                for d in range(ND):
                                    nc.tensor.matmul(
                                        up[:, :cl], w1f[:, d, :],
                                        h2Ts[d][:, c0:c0 + cl],
                                        start=(d == 0), stop=(d == ND - 1))
                                nc.scalar.activation(
                                    out=Us[fi][:, c0:c0 + cl], in_=up[:, :cl],
                                    func=RELU, bias=b1c[:, f:f + 1], scale=1.0)
                        for db in range(D // 512):
                            w2s = []
                            for fi in range(FGN):
                                f = g * FGN + fi
                                w2t = pw2.tile([P, 512], BF16, tag=f"w2s{fi}", name=f"w2s{fi}")
                                nc.sync.dma_start(out=w2t, in_=bass.AP(
                                    tensor=W2.ap().tensor,
                                    offset=f * P * D + db * 512,
                                    ap=[[D, P], [1, 512]]))
                                w2s.append(w2t)
                            for t in range(NQ):
                                op = psO.tile([P, 512], F32, tag="op")
                                for fi in range(FGN):
                                    nc.tensor.matmul(
                                        op, Us[fi][:, t * P:(t + 1) * P], w2s[fi],
                                        start=(fi == 0), stop=(fi == FGN - 1))
                                fb = pcb.tile([P, 512], F32, tag="fb")
                                if g < NG - 1:
                                    nc.vector.tensor_copy(fb, op)
                                    nc.sync.dma_start(
                                        out=ff_dram[g][t * P:(t + 1) * P,
                                                       db * 512:(db + 1) * 512],
                                        in_=fb)
                                else:
                                    x2c = pcb.tile([P, 512], F32, tag="x2c")
                                    nc.sync.dma_start(
                                        out=x2c,
                                        in_=x2_dram[t * P:(t + 1) * P,
                                                    db * 512:(db + 1) * 512])
                                    nc.vector.tensor_add(out=fb, in0=op, in1=x2c)
                                    for gg in range(NG - 1):
                                        fgc = pcb.tile([P, 512], F32,
                                                       tag=f"fgc{gg}",
                                                       name=f"fgc{gg}")
                                        nc.sync.dma_start(
                                            out=fgc,
                                            in_=ff_dram[gg][t * P:(t + 1) * P,
                                                            db * 512:(db + 1) * 512])
                                        nc.vector.tensor_add(out=fb, in0=fb, in1=fgc)
                                    nc.vector.tensor_add(
                                        out=fb, in0=fb,
                                        in1=b2b[:, db * 512:(db + 1) * 512])
                                    nc.sync.dma_start(
                                        out=out.ap()[t * P:(t + 1) * P,
                                                     db * 512:(db + 1) * 512],
                                        in_=fb)

    nc.compile()
    return nc


_NC_CACHE = {}


def get_nc(key="full"):
    if key not in _NC_CACHE:
        _NC_CACHE[key] = build_nc(FULL if key == "full" else SMALL)
    return _NC_CACHE[key]


def make_in_maps(inputs, cfg):
    T, D, H, FF = cfg["T"], cfg["D"], cfg["H"], cfg["FF"]
    x = np.asarray(inputs["x"], np.float32)
    B = x.shape[0]
    bf = ml_dtypes.bfloat16
    shared = {
        "Wq": np.asarray(inputs["Wq"], np.float32).astype(bf),
        "Wk": np.asarray(inputs["Wk"], np.float32).astype(bf),
        "Wv": np.asarray(inputs["Wv"], np.float32).astype(bf),
        "bq": np.asarray(inputs["bq"], np.float32),
        "bk": np.asarray(inputs["bk"], np.float32),
        "bv": np.asarray(inputs["bv"], np.float32),
        "W1": np.asarray(inputs["W1"], np.float32),
        "b1": np.asarray(inputs["b1"], np.float32),
        "W2": np.asarray(inputs["W2"], np.float32).astype(bf),
        "b2": np.asarray(inputs["b2"], np.float32),
        "g1": np.asarray(inputs["g1"], np.float32),
        "be1": np.asarray(inputs["be1"], np.float32),
        "g2": np.asarray(inputs["g2"], np.float32),
        "be2": np.asarray(inputs["be2"], np.float32),
    }
    in_maps = []
    n_cores = 2 * B
    for c in range(n_cores):
        b, p = c // 2, c % 2
        rows = np.concatenate([np.arange(g * P, (g + 1) * P)
                               for g in range(p, T // P, 2)])
        cols = np.arange(256)[None, :]
        r = np.arange(P)[:, None]
        m = np.where(cols > P * p + r, np.float32(-1e9), np.float32(0.0))
        im = dict(shared)
        im["x_ctx"] = x[b]
        im["xq"] = x[b][rows]
        im["mask"] = m
        in_maps.append(im)
    return in_maps


def assemble(results, cfg, B):
    T, D = cfg["T"], cfg["D"]
    out = np.zeros((B, T, D), np.float32)
    for c in range(2 * B):
        b, p = c // 2, c % 2
        rows = np.concatenate([np.arange(g * P, (g + 1) * P)
                               for g in range(p, T // P, 2)])
        out[b][rows] = results[c]["out"]
    return out


def run(inputs, cfg=FULL, key="full", trace=False, **kw):
    nc = get_nc(key)
    in_maps = make_in_maps(inputs, cfg)
    res = bass_utils.run_bass_kernel_spmd(
        nc, in_maps, core_ids=list(range(len(in_maps))), trace=trace, **kw)
    B = np.asarray(inputs["x"]).shape[0]
    return assemble(res.results, cfg, B), res


def kernel(**inputs):
    out, _ = run(inputs)
    return out



# revision 12
# speedup vs baseline: 1.2471x; 1.2471x over previous
"""AttentionBlock kernel for 8 Trainium2 NeuronCores.

Sharding: core c -> batch b = c//2, parity p = c%2. Within a batch pair the
K/V projections are split by heads (core parity p computes K/V for global
heads 8p..8p+8 over the full context) and exchanged via pairwise AllGather
collectives; attention + FFN are split by query rows (parity-interleaved
128-row tiles, which balances causal attention work). All rank asymmetry
enters through input data (row shards, weight shards, masks) so one SPMD
program serves all 8 cores.

Math/dtypes:
- LN gains/biases are folded into the projection weights/biases host-side.
- Q/K/V projections and A@V run in fp8(e4m3) DoubleRow matmuls (2x rate,
  256-deep contraction per instruction) with per-tensor scaling.
- Logits are computed transposed (S^T = K Q^T) so the attention probs need
  no transpose before A@V; the softmax denominator comes from a ones-column
  appended to V.
- QK^T in bf16; FFN in bf16; PSUM accumulation fp32 everywhere.
"""
import sys
sys.path.insert(0, "/opt/trn_rl_repo")

import numpy as np
import ml_dtypes

import concourse.bacc as bacc
import concourse.bass as bass
import concourse.mybir as mybir
import concourse.tile as tile
from concourse import bass_utils
from concourse.masks import make_identity

P = 128
F32 = mybir.dt.float32
BF16 = mybir.dt.bfloat16
FP8 = mybir.dt.float8e4

T, D, H, FF, HD = 2048, 2048, 16, 8192, 128
NT = T // P            # 16 context tiles
NQ = NT // 2           # 8 query slots per core
TQ = NQ * P            # 1024 query rows per core
ND = D // P            # 16 d tiles
NDP = ND // 2          # 8 d pairs
NF = FF // P           # 64 ffn hidden tiles
HL = H // 2            # 8 local heads per core
HG = 4                 # heads per V-production group
SCALE = 1.0 / np.sqrt(HD)
EPS = 1e-5

# fp8 scaling factors
S_H = 16.0             # ln1 output scale
S_W = 128.0            # qkv weight scale
S_P = 1.0 / 16.0       # attention prob scale (exp bias = ln S_P); max
                       # observed logit ~7.9 -> e^7.9/16 = 165 < fp8 max 240
S_V = 16.0             # v scale; ones column holds S_V so scales cancel

RG = [[0, 1], [2, 3], [4, 5], [6, 7]]
DR = mybir.MatmulPerfMode.DoubleRow


def build_nc():
    nc = bacc.Bacc("TRN2", target_bir_lowering=False)

    x_ctx = nc.dram_tensor("x_ctx", [T, D], F32, kind="ExternalInput")
    xq = nc.dram_tensor("xq", [TQ, D], F32, kind="ExternalInput")
    # folded+scaled fp8 weights: wq8 all 16 heads; wk8/wv8 my 8 heads
    wq8 = nc.dram_tensor("wq8", [H, D, HD], FP8, kind="ExternalInput")
    wk8 = nc.dram_tensor("wk8", [HL, D, HD], FP8, kind="ExternalInput")
    wv8 = nc.dram_tensor("wv8", [HL, D, HD], FP8, kind="ExternalInput")
    bqf = nc.dram_tensor("bqf", [H, HD], F32, kind="ExternalInput")
    bkf = nc.dram_tensor("bkf", [HL, HD], F32, kind="ExternalInput")
    bvf = nc.dram_tensor("bvf", [HL, HD], F32, kind="ExternalInput")   # pre-scaled x2048
    W1 = nc.dram_tensor("W1", [D, FF], BF16, kind="ExternalInput")     # g2-folded
    b1 = nc.dram_tensor("b1", [FF], F32, kind="ExternalInput")         # be2-folded
    W2 = nc.dram_tensor("W2", [FF, D], BF16, kind="ExternalInput")
    b2 = nc.dram_tensor("b2", [D], F32, kind="ExternalInput")
    maskT = nc.dram_tensor("maskT", [P, 2, P], F32, kind="ExternalInput")
    out = nc.dram_tensor("out", [TQ, D], F32, kind="ExternalOutput")

    ID = mybir.ActivationFunctionType.Identity
    EXP = mybir.ActivationFunctionType.Exp
    SQRT = mybir.ActivationFunctionType.Sqrt
    AL = mybir.AluOpType

    with tile.TileContext(nc) as tc:
        with tc.tile_pool(name="consts", bufs=1) as consts, \
             tc.tile_pool(name="dram", bufs=1, space="DRAM") as dpool:
            ident8 = consts.tile([P, P], FP8, tag="ident8")
            make_identity(nc, ident8)
            identb = consts.tile([P, P], BF16, tag="identb")
            make_identity(nc, identb)
            eps256 = consts.tile([P, 1], F32, tag="eps256")
            nc.gpsimd.memset(eps256, EPS / 256.0)
            eps_t = consts.tile([P, 1], F32, tag="epst")
            nc.gpsimd.memset(eps_t, EPS)
            lnsp = consts.tile([P, 1], F32, tag="lnsp")
            nc.gpsimd.memset(lnsp, float(np.log(S_P)))
            c_deq = consts.tile([P, 1], F32, tag="cdeq")
            nc.gpsimd.memset(c_deq, 1.0 / (S_H * S_W))
            c_v = consts.tile([P, 1], F32, tag="cv")
            nc.gpsimd.memset(c_v, 1.0 / S_W)
            zero_t = consts.tile([P, 1], F32, tag="zt")
            nc.gpsimd.memset(zero_t, 0.0)
            # biases
            bqc = consts.tile([P, H], F32, tag="bqc")
            nc.sync.dma_start(out=bqc, in_=bass.AP(
                tensor=bqf.ap().tensor, offset=0, ap=[[1, P], [HD, H]]))
            bkc = consts.tile([P, HL], F32, tag="bkc")
            nc.sync.dma_start(out=bkc, in_=bass.AP(
                tensor=bkf.ap().tensor, offset=0, ap=[[1, P], [HD, HL]]))
            # bvf broadcast per head-group: [128, HG*HD]
            bvb = [consts.tile([P, HG * HD], F32, tag=f"bvb{g}", name=f"bvb{g}")
                   for g in range(HL // HG)]
            for g in range(HL // HG):
                nc.sync.dma_start(out=bvb[g], in_=bass.AP(
                    tensor=bvf.ap().tensor, offset=g * HG * HD,
                    ap=[[0, P], [1, HG * HD]]))
            b1c = consts.tile([P, NF], F32, tag="b1c")
            nc.sync.dma_start(out=b1c, in_=bass.AP(
                tensor=b1.ap().tensor, offset=0, ap=[[1, P], [P, NF]]))
            b2b = consts.tile([P, D], F32, tag="b2b")
            nc.sync.dma_start(out=b2b, in_=bass.AP(
                tensor=b2.ap().tensor, offset=0, ap=[[0, P], [1, D]]))
            mskT = consts.tile([P, 2, P], F32, tag="mskT")
            nc.sync.dma_start(out=mskT, in_=maskT.ap())

            # DRAM: collective bounce buffers + x2 spill
            ccK_in = [dpool.tile([HL // 2, P, T], BF16, tag=f"cKi{a}", name=f"cKi{a}")
                      for a in range(2)]
            ccK_out = [dpool.tile([2, HL // 2, P, T], BF16, tag=f"cKo{a}", name=f"cKo{a}")
                       for a in range(2)]
            ccV_in = [dpool.tile([HL // 2, T, HD], FP8, tag=f"cVi{a}", name=f"cVi{a}")
                      for a in range(2)]
            ccV_out = [dpool.tile([2, HL // 2, T, HD], FP8, tag=f"cVo{a}", name=f"cVo{a}")
                       for a in range(2)]
            x2_dram = dpool.tile([TQ, D], F32, tag="x2d")

            with tc.tile_pool(name="h2T", bufs=1) as h2p:
                h2T = h2p.tile([P, ND, TQ], BF16, tag="h2T")

                with tc.tile_pool(name="qt", bufs=1) as qtp, \
                     tc.tile_pool(name="attn", bufs=1) as atp:
                    QT = qtp.tile([P, H, TQ], BF16, tag="QT")
                    attn_sb = atp.tile([P, NQ, D], BF16, tag="attn_sb")

                    # ============ Phase A: LN1 -> hT fp8 pairs ============
                    with tc.tile_pool(name="hT", bufs=1) as hTp:
                        hTs = [hTp.tile([P, 2, T + TQ], FP8, tag=f"hT{k}",
                                        name=f"hT{k}") for k in range(NDP)]
                        with tc.tile_pool(name="phA", bufs=3) as pa, \
                             tc.tile_pool(name="phA2", bufs=2) as pa2, \
                             tc.tile_pool(name="psA", bufs=4, space="PSUM") as psA:
                            def ln1_tile(src_ap, dst_col, eng_alt):
                                xin = pa.tile([P, D], F32, tag="xin")
                                nc.sync.dma_start(out=xin, in_=src_ap)
                                st = pa.tile([P, 4, 6], F32, tag="st")
                                xr = xin.rearrange("p (n f) -> p n f", n=4)
                                for s in range(4):
                                    nc.vector.bn_stats(out=st[:, s, :], in_=xr[:, s, :])
                                mv = pa.tile([P, 2], F32, tag="mv")
                                nc.vector.bn_aggr(out=mv, in_=st)
                                # rstd16 = 16/sqrt(var+eps) = 1/sqrt((var+eps)/256)
                                rstd = pa.tile([P, 1], F32, tag="rstd")
                                nc.scalar.activation(out=rstd, in_=mv[:, 1:2],
                                                     func=SQRT, bias=eps256,
                                                     scale=1.0 / 256.0)
                                nc.vector.reciprocal(out=rstd, in_=rstd)
                                hb = pa2.tile([P, D], FP8, tag="hb")
                                nc.vector.tensor_scalar(
                                    out=hb, in0=xin, scalar1=mv[:, 0:1],
                                    scalar2=rstd, op0=AL.subtract, op1=AL.mult)
                                for d in range(ND):
                                    # fp8 PE transpose needs output elem step 2
                                    tp = psA.tile([P, P, 2], FP8, tag="tpA")
                                    nc.tensor.transpose(
                                        tp[:, :, 0], hb[:, d * P:(d + 1) * P],
                                        ident8)
                                    dst = hTs[d // 2][:, d % 2,
                                                      dst_col:dst_col + P]
                                    if eng_alt:
                                        nc.scalar.activation(
                                            out=dst, in_=tp[:, :, 0], func=ID,
                                            bias=zero_t, scale=1.0)
                                    else:
                                        nc.vector.tensor_copy(dst, tp[:, :, 0])

                            xr_ctx = x_ctx.ap().rearrange("(n p) d -> n p d", p=P)
                            for g in range(NT):
                                ln1_tile(xr_ctx[g], g * P, g % 2 == 0)
                            xr_q = xq.ap().rearrange("(n p) d -> n p d", p=P)
                            for j in range(NQ):
                                ln1_tile(xr_q[j], T + j * P, j % 2 == 0)

                        # ============ Phase B1: K/V (8 local heads) + Q ====
                        with tc.tile_pool(name="phB1", bufs=2) as pb1, \
                             tc.tile_pool(name="phB1v", bufs=2) as pbv, \
                             tc.tile_pool(name="ps512", bufs=4, space="PSUM") as ps512:
                            def qkv_w_ap(wt, h):
                                # [H?, D, HD] fp8 dram -> sbuf [128, NDP, 2, HD]
                                return bass.AP(
                                    tensor=wt.ap().tensor, offset=h * D * HD,
                                    ap=[[HD, P], [2 * P * HD, NDP],
                                        [P * HD, 2], [1, HD]])

                            def proj_psum(pp, wtile, cols, n):
                                # pp[e, n] += sum_d w[d,e] hT[d, cols:cols+n]
                                for k in range(NDP):
                                    nc.tensor.matmul(
                                        pp[:, :n], wtile[:, k, :, :],
                                        hTs[k][:, :, cols:cols + n],
                                        start=(k == 0), stop=(k == NDP - 1),
                                        perf_mode=DR)

                            def halfheads(a):
                                # K for local heads [4a, 4a+4)
                                for hl in range(4 * a, 4 * a + 4):
                                    wk_t = pb1.tile([P, NDP, 2, HD], FP8, tag="wk")
                                    nc.sync.dma_start(out=wk_t, in_=qkv_w_ap(wk8, hl))
                                    KT = pb1.tile([P, T], BF16, tag="KT")
                                    for c in range(T // 512):
                                        pk = ps512.tile([P, 512], F32, tag="p512")
                                        proj_psum(pk, wk_t, c * 512, 512)
                                        nc.vector.tensor_scalar(
                                            out=KT[:, c * 512:(c + 1) * 512],
                                            in0=pk, scalar1=c_deq,
                                            scalar2=bkc[:, hl:hl + 1],
                                            op0=AL.mult, op1=AL.add)
                                    nc.sync.dma_start(
                                        out=ccK_in[a][hl - 4 * a], in_=KT)
                                # V for the same 4 heads, produced s-block-major
                                wv4 = pbv.tile([P, NDP, 2, HG * HD], FP8, tag="wv4")
                                for q in range(HG):
                                    nc.sync.dma_start(
                                        out=wv4[:, :, :, q * HD:(q + 1) * HD],
                                        in_=qkv_w_ap(wv8, 4 * a + q))
                                V4 = pbv.tile([P, NT, HG * HD], FP8, tag="V4")
                                for s in range(NT):
                                    pv = ps512.tile([P, HG * HD], F32, tag="p512")
                                    for k in range(NDP):
                                        nc.tensor.matmul(
                                            pv, hTs[k][:, :, s * P:(s + 1) * P],
                                            wv4[:, k, :, :],
                                            start=(k == 0), stop=(k == NDP - 1),
                                            perf_mode=DR)
                                    nc.vector.tensor_tensor(
                                        out=pv, in0=pv, in1=bvb[a], op=AL.add)
                                    nc.scalar.activation(
                                        out=V4[:, s, :], in_=pv, func=ID,
                                        bias=zero_t, scale=1.0 / S_W)
                                for q in range(HG):
                                    # V4[sp, s, q*HD:+HD] -> ccV_in[a][q][s*128+sp, e]
                                    dst = bass.AP(
                                        tensor=ccV_in[a][:].tensor,
                                        offset=ccV_in[a][:].offset + q * T * HD,
                                        ap=[[HD, P], [P * HD, NT], [1, HD]])
                                    nc.sync.dma_start(
                                        out=dst, in_=V4[:, :, q * HD:(q + 1) * HD])
                                nc.gpsimd.collective_compute(
                                    "AllGather", AL.bypass, replica_groups=RG,
                                    ins=[ccK_in[a][:].opt()],
                                    outs=[ccK_out[a][:].opt()])
                                nc.gpsimd.collective_compute(
                                    "AllGather", AL.bypass, replica_groups=RG,
                                    ins=[ccV_in[a][:].opt()],
                                    outs=[ccV_out[a][:].opt()])

                            halfheads(0)
                            halfheads(1)
                            # Q for all 16 global heads over my TQ rows
                            for h in range(H):
                                wq_t = pb1.tile([P, NDP, 2, HD], FP8, tag="wq")
                                nc.sync.dma_start(out=wq_t, in_=qkv_w_ap(wq8, h))
                                for c in range(TQ // 512):
                                    pq = ps512.tile([P, 512], F32, tag="p512")
                                    proj_psum(pq, wq_t, T + c * 512, 512)
                                    nc.vector.tensor_scalar(
                                        out=QT[:, h, c * 512:(c + 1) * 512],
                                        in0=pq, scalar1=c_deq,
                                        scalar2=bqc[:, h:h + 1],
                                        op0=AL.mult, op1=AL.add)

                    # ============ Phase B2: attention (16 global heads) ====
                    with tc.tile_pool(name="phB2", bufs=2) as pb2, \
                         tc.tile_pool(name="phB2s", bufs=2) as pbs, \
                         tc.tile_pool(name="psL", bufs=4, space="PSUM") as psL, \
                         tc.tile_pool(name="psAV", bufs=2, space="PSUM") as psAV:
                        head_order = [0, 1, 2, 3, 8, 9, 10, 11,
                                      4, 5, 6, 7, 12, 13, 14, 15]
                        for hh in head_order:
                            a, chunk, idx = (0 if hh % 8 < 4 else 1,
                                             hh // 8, hh % 4)
                            KTh = pb2.tile([P, T], BF16, tag="KTh")
                            nc.sync.dma_start(
                                out=KTh, in_=ccK_out[a][chunk, idx])
                            Vh = pb2.tile([P, NQ, 2, HD + 1], FP8, tag="Vh")
                            nc.gpsimd.memset(Vh[:, :, :, HD:HD + 1], S_V)
                            nc.sync.dma_start(
                                out=Vh[:, :, :, 0:HD],
                                in_=bass.AP(
                                    tensor=ccV_out[a].tensor,
                                    offset=(chunk * (HL // 2) + idx) * T * HD,
                                    ap=[[HD, P], [2 * P * HD, NQ],
                                        [P * HD, 2], [1, HD]]))
                            SPs = [pbs.tile([P, 2, (NQ - k) * P], FP8,
                                            tag=f"SP{k}", name=f"SP{k}")
                                   for k in range(NQ)]
                            for k in range(NQ):
                                w = (NQ - k) * P   # t-cols [k*128, 1024)
                                for i in range(2):
                                    s0 = (2 * k + i) * P
                                    nch = (w + 511) // 512
                                    for c in range(nch):
                                        cw = min(512, w - c * 512)
                                        lp = psL.tile([P, 512], F32, tag="lp")
                                        nc.tensor.matmul(
                                            lp[:, :cw], KTh[:, s0:s0 + P],
                                            QT[:, hh, k * P + c * 512:
                                               k * P + c * 512 + cw],
                                            start=True, stop=True)
                                        if c == 0:
                                            nc.vector.tensor_tensor(
                                                out=lp[:, :P], in0=lp[:, :P],
                                                in1=mskT[:, i, :], op=AL.add)
                                        nc.scalar.activation(
                                            out=SPs[k][:, i, c * 512:
                                                       c * 512 + cw],
                                            in_=lp[:, :cw], func=EXP,
                                            bias=lnsp, scale=SCALE)
                            for j in range(NQ):
                                av = psAV.tile([P, HD + 1], F32, tag="av")
                                for k in range(j + 1):
                                    nc.tensor.matmul(
                                        av,
                                        SPs[k][:, :, (j - k) * P:(j - k + 1) * P],
                                        Vh[:, k, :, :],
                                        start=(k == 0), stop=(k == j),
                                        perf_mode=DR)
                                rs = pbs.tile([P, 1], F32, tag="rs")
                                nc.vector.reciprocal(out=rs, in_=av[:, HD:HD + 1])
                                nc.vector.tensor_scalar_mul(
                                    out=attn_sb[:, j, hh * HD:(hh + 1) * HD],
                                    in0=av[:, 0:HD], scalar1=rs)

                    # ============ Phase C: residual + LN2 -> h2T ============
                    with tc.tile_pool(name="phC", bufs=3) as pc, \
                         tc.tile_pool(name="phC2", bufs=2) as pc2, \
                         tc.tile_pool(name="psC", bufs=4, space="PSUM") as psC:
                        xr_q = xq.ap().rearrange("(n p) d -> n p d", p=P)
                        for t in range(NQ):
                            xt = pc.tile([P, D], F32, tag="xt")
                            nc.sync.dma_start(out=xt, in_=xr_q[t])
                            x2 = pc.tile([P, D], F32, tag="x2t")
                            nc.vector.tensor_tensor(
                                out=x2, in0=xt, in1=attn_sb[:, t, :], op=AL.add)
                            nc.sync.dma_start(
                                out=x2_dram[t * P:(t + 1) * P, :], in_=x2)
                            st = pc.tile([P, 4, 6], F32, tag="st2")
                            x2r = x2.rearrange("p (n f) -> p n f", n=4)
                            for s in range(4):
                                nc.vector.bn_stats(out=st[:, s, :], in_=x2r[:, s, :])
                            mv = pc.tile([P, 2], F32, tag="mv2")
                            nc.vector.bn_aggr(out=mv, in_=st)
                            rstd = pc.tile([P, 1], F32, tag="rstd2")
                            nc.scalar.activation(out=rstd, in_=mv[:, 1:2],
                                                 func=SQRT, bias=eps_t, scale=1.0)
                            nc.vector.reciprocal(out=rstd, in_=rstd)
                            h2 = pc2.tile([P, D], BF16, tag="h2tmp")
                            nc.vector.tensor_scalar(
                                out=h2, in0=x2, scalar1=mv[:, 0:1],
                                scalar2=rstd, op0=AL.subtract, op1=AL.mult)
                            for d in range(ND):
                                tp = psC.tile([P, P], BF16, tag="tpC")
                                nc.tensor.transpose(
                                    tp, h2[:, d * P:(d + 1) * P], identb)
                                if d % 2 == 0:
                                    nc.scalar.activation(
                                        out=h2T[:, d, t * P:(t + 1) * P],
                                        in_=tp, func=ID, bias=zero_t, scale=1.0)
                                else:
                                    nc.vector.tensor_copy(
                                        h2T[:, d, t * P:(t + 1) * P], tp)

                # ============ FFN (qt/attn pools freed) ============
                with tc.tile_pool(name="us", bufs=1) as usp, \
                     tc.tile_pool(name="phW1", bufs=3) as pw1, \
                     tc.tile_pool(name="phW2", bufs=3) as pw2, \
                     tc.tile_pool(name="phE", bufs=2) as pe:
                    Us = usp.tile([P, NF, TQ], BF16, tag="Us")
                    with tc.tile_pool(name="psU", bufs=4, space="PSUM") as psU:
                        for f in range(NF):
                            w1f = pw1.tile([P, ND, P], BF16, tag="w1f")
                            nc.sync.dma_start(out=w1f, in_=bass.AP(
                                tensor=W1.ap().tensor, offset=f * P,
                                ap=[[FF, P], [P * FF, ND], [1, P]]))
                            for c in range(TQ // 512):
                                up = psU.tile([P, 512], F32, tag="up")
                                for d in range(ND):
                                    nc.tensor.matmul(
                                        up, w1f[:, d, :],
                                        h2T[:, d, c * 512:(c + 1) * 512],
                                        start=(d == 0), stop=(d == ND - 1))
                                nc.vector.tensor_scalar(
                                    out=Us[:, f, c * 512:(c + 1) * 512],
                                    in0=up, scalar1=b1c[:, f:f + 1],
                                    scalar2=zero_t, op0=AL.add, op1=AL.max)
                    with tc.tile_pool(name="psO", bufs=1, space="PSUM") as psO:
                      for db in range(D // 512):
                        ops = [psO.tile([P, 512], F32, tag=f"op{t}",
                                        name=f"op{t}") for t in range(NQ)]
                        for f in range(NF):
                            w2t = pw2.tile([P, 512], BF16, tag="w2t")
                            nc.sync.dma_start(out=w2t, in_=bass.AP(
                                tensor=W2.ap().tensor,
                                offset=f * P * D + db * 512,
                                ap=[[D, P], [1, 512]]))
                            for t in range(NQ):
                                nc.tensor.matmul(
                                    ops[t], Us[:, f, t * P:(t + 1) * P], w2t,
                                    start=(f == 0), stop=(f == NF - 1))
                        for t in range(NQ):
                            x2c = pe.tile([P, 512], F32, tag="x2c")
                            nc.sync.dma_start(
                                out=x2c,
                                in_=x2_dram[t * P:(t + 1) * P,
                                            db * 512:(db + 1) * 512])
                            fb = pe.tile([P, 512], F32, tag="fb")
                            nc.vector.tensor_tensor(
                                out=fb, in0=ops[t], in1=x2c, op=AL.add)
                            nc.vector.tensor_tensor(
                                out=fb, in0=fb,
                                in1=b2b[:, db * 512:(db + 1) * 512], op=AL.add)
                            nc.sync.dma_start(
                                out=out.ap()[t * P:(t + 1) * P,
                                             db * 512:(db + 1) * 512],
                                in_=fb)

    nc.compile()
    return nc


_NC_CACHE = {}


def get_nc():
    if "full" not in _NC_CACHE:
        _NC_CACHE["full"] = build_nc()
    return _NC_CACHE["full"]


def make_in_maps(inputs):
    f32 = np.float32
    fp8 = ml_dtypes.float8_e4m3
    bf = ml_dtypes.bfloat16
    x = np.asarray(inputs["x"], f32)
    B = x.shape[0]
    Wq = np.asarray(inputs["Wq"], f32)
    Wk = np.asarray(inputs["Wk"], f32)
    Wv = np.asarray(inputs["Wv"], f32)
    bq = np.asarray(inputs["bq"], f32)
    bk = np.asarray(inputs["bk"], f32)
    bv = np.asarray(inputs["bv"], f32)
    g1 = np.asarray(inputs["g1"], f32)
    be1 = np.asarray(inputs["be1"], f32)
    g2 = np.asarray(inputs["g2"], f32)
    be2 = np.asarray(inputs["be2"], f32)
    W1 = np.asarray(inputs["W1"], f32)
    b1 = np.asarray(inputs["b1"], f32)
    W2 = np.asarray(inputs["W2"], f32)
    b2 = np.asarray(inputs["b2"], f32)

    # fold LN1 gains into qkv weights, biases into qkv biases
    wq8 = (g1[None, :, None] * Wq * S_W).astype(fp8)
    wk8 = (g1[None, :, None] * Wk * S_W).astype(fp8)
    wv8 = (g1[None, :, None] * Wv * S_W).astype(fp8)
    bqf = (bq + np.einsum("d,hde->he", be1, Wq)).astype(f32)
    bkf = (bk + np.einsum("d,hde->he", be1, Wk)).astype(f32)
    bvf = ((bv + np.einsum("d,hde->he", be1, Wv)) * (S_H * S_W)).astype(f32)
    # fold LN2 gains into W1
    W1f = (g2[:, None] * W1).astype(bf)
    b1f = (b1 + be2 @ W1).astype(f32)
    W2b = W2.astype(bf)

    shared = {
        "wq8": wq8, "bqf": bqf, "W1": W1f, "b1": b1f,
        "W2": W2b, "b2": b2,
    }
    in_maps = []
    for c in range(2 * B):
        b, p = c // 2, c % 2
        rows = np.concatenate([np.arange(g * P, (g + 1) * P)
                               for g in range(p, NT, 2)])
        # maskT[sp, i, tp]: -1e9 where (i-p)*128 + sp > tp
        sp = np.arange(P)[:, None, None]
        i = np.arange(2)[None, :, None]
        tp = np.arange(P)[None, None, :]
        m = np.where((i - p) * P + sp > tp, f32(-1e9), f32(0.0))
        im = dict(shared)
        im["x_ctx"] = x[b]
        im["xq"] = x[b][rows]
        im["wk8"] = wk8[HL * p:HL * (p + 1)]
        im["wv8"] = wv8[HL * p:HL * (p + 1)]
        im["bkf"] = bkf[HL * p:HL * (p + 1)]
        im["bvf"] = bvf[HL * p:HL * (p + 1)]
        im["maskT"] = np.ascontiguousarray(m)
        in_maps.append(im)
    return in_maps


def assemble(results, B):
    out = np.zeros((B, T, D), np.float32)
    for c in range(2 * B):
        b, p = c // 2, c % 2
        rows = np.concatenate([np.arange(g * P, (g + 1) * P)
                               for g in range(p, NT, 2)])
        out[b][rows] = results[c]["out"]
    return out


def run(inputs, trace=False, **kw):
    nc = get_nc()
    in_maps = make_in_maps(inputs)
    res = bass_utils.run_bass_kernel_spmd(
        nc, in_maps, core_ids=list(range(len(in_maps))), trace=trace, **kw)
    B = np.asarray(inputs["x"]).shape[0]
    return assemble(res.results, B), res


def kernel(**inputs):
    out, _ = run(inputs)
    return out
